# revision 3
# baseline (speedup 1.0000x reference)
import sys

for p in ("/opt/trn_rl_repo",):
    if p not in sys.path:
        sys.path.insert(0, p)

import numpy as np
import ml_dtypes

from concourse import bass, mybir, bacc, tile
from concourse.ap import AP
from concourse.bass_utils import run_bass_kernel_spmd


def _install_ntff_hook():
    try:
        from antenv import axon_hooks  # noqa: F401
        return
    except ImportError:
        pass
    import types
    try:
        import antenv
    except ImportError:
        return
    mod = types.ModuleType("antenv.axon_hooks")
    _h = {"hook": None}
    mod.set_axon_ntff_profile_hook = lambda h: _h.__setitem__("hook", h)
    mod.get_axon_ntff_profile_hook = lambda: _h["hook"]
    sys.modules["antenv.axon_hooks"] = mod
    antenv.axon_hooks = mod
    try:
        from trn_agent_boot.trn_boot import _ntff_profile_via_ctypes
        h = _ntff_profile_via_ctypes("/opt/axon/libaxon_pjrt.so")
        if h is not None:
            mod.set_axon_ntff_profile_hook(h)
    except Exception:
        pass


_install_ntff_hook()


def _enable_ldw_opt():
    import concourse.bass_utils as _bu
    if getattr(_bu, "_ldw_patched", False):
        return
    _orig = _bu.run_command

    def _patched(argv, **kw):
        try:
            argv = ["--enable-ldw-opt=true" if c == "--enable-ldw-opt=false" else c
                    for c in argv]
        except TypeError:
            pass
        return _orig(argv, **kw)

    _bu.run_command = _patched
    _bu._ldw_patched = True


# _enable_ldw_opt() breaks walrus codegen (InstLdweights incompatible)

F32 = mybir.dt.float32
BF16 = mybir.dt.bfloat16
F8 = mybir.dt.float8e4
MUL = mybir.AluOpType.mult
ADD = mybir.AluOpType.add
AXX = mybir.AxisListType.X
EXP = mybir.ActivationFunctionType.Exp

B, C, H, W = 16, 256, 96, 96
S = H * W
NB = 32
NCORE = 8
BPC = B // NCORE
QKW = 66            # ch0..31 q, 32 ones, 33..64 k, 65 sigma (no pad)
PW = QKW + 256
NCHT = 66           # transposed channels 0..65

V_DT = F8           # value matrix storage dtype
RHS_DT = F8         # rhs (attention pattern) dtype — must match v dtype for PE


def _apv(t, off, dims):
    b = t[:] if not isinstance(t, AP) else t
    part = list(b.ap[0])
    return AP(b.tensor, b.offset + off, [part] + [list(d) for d in dims])


def _apvp(t, p0, np_, off, dims):
    """View with custom partition range [p0, p0+np_) and custom free dims."""
    b = t[:] if not isinstance(t, AP) else t
    part = [b.ap[0][0], np_]
    return AP(b.tensor, b.offset + p0 * b.ap[0][0] + off,
              [part] + [list(d) for d in dims])


def build_graph():
    nc = bacc.Bacc(None, target_bir_lowering=False)

    xa_e = nc.declare_dram_parameter("xa", [BPC, 2, 128, S], BF16, isOutput=False)
    wall_e = nc.declare_dram_parameter("wall", [2, 128, PW], BF16, isOutput=False)
    ipat2_e = nc.declare_dram_parameter("ipat2", [96, 1728], BF16, isOutput=False)
    pstr_e = nc.declare_dram_parameter("pstr", [96, 96], BF16, isOutput=False)
    idtb_e = nc.declare_dram_parameter("idtb", [96, 96], BF16, isOutput=False)
    bvrow_e = nc.declare_dram_parameter("bvrow", [1, 96 * 256], V_DT, isOutput=False)
    out_e = nc.declare_dram_parameter("out", [BPC, 2, 128, S], BF16, isOutput=True)

    with tile.TileContext(nc) as tc, nc.allow_low_precision(reason="bf16 score accum ok for 2e-2 tol"):
        with (
            tc.tile_pool(name="const", bufs=1) as cp,
            tc.tile_pool(name="main", bufs=1) as mp,
            tc.tile_pool(name="work", bufs=2) as wp,
            tc.tile_pool(name="pj", bufs=4, space="PSUM") as pj,
            tc.tile_pool(name="avp", bufs=4, space="PSUM") as avp,
        ):
            wall_sb = []
            for cc in range(2):
                t = cp.tile([128, PW], BF16, tag=f"wall{cc}")
                nc.sync.dma_start(t[:], wall_e[cc])
                wall_sb.append(t)
            ipat2_sb = cp.tile([96, 1728], BF16, tag="ipat2")
            nc.sync.dma_start(ipat2_sb[:], ipat2_e[:])
            pstr_sb = cp.tile([96, 96], BF16, tag="pstr")
            nc.sync.dma_start(pstr_sb[:], pstr_e[:])
            idtb_sb = cp.tile([96, 96], BF16, tag="idtb")
            nc.sync.dma_start(idtb_sb[:], idtb_e[:])

            st = {}

            def stage_load(b):
                xs = []
                for cc in range(2):
                    t = mp.tile([128, S], BF16, tag=f"xa{cc}", bufs=2,
                                name=f"xa{cc}_{b}")
                    xs.append(t)
                # interleave cc chunks so early proj groups unblock fast
                for q in range(4):
                    for cc in range(2):
                        nc.sync.dma_start(xs[cc][:, q * 2304:(q + 1) * 2304],
                                          xa_e[b, cc, :, q * 2304:(q + 1) * 2304])
                v = mp.tile([97, 96 * 256], V_DT, tag="v", bufs=2, name=f"v{b}")
                nc.sync.dma_start(v[96:97, :], bvrow_e[:])
                qk = mp.tile([96, 96 * QKW], BF16, tag="qk", bufs=2, name=f"qk{b}")
                qkt = mp.tile([96, 96 * NCHT], BF16, tag="qkt", bufs=2,
                              name=f"qkt{b}")
                st[b] = {"xs": xs, "v": v, "qk": qk, "qkt": qkt}

            def stage_proj_group(b, g):
                # 1 line per group, g in 0..95
                xs, v, qk = st[b]["xs"], st[b]["v"], st[b]["qk"]
                ps = pj.tile([96, 512], F32, tag="ps", name=f"ps{b}_{g}")
                h = g
                for cc in range(2):
                    nc.tensor.matmul(
                        _apv(ps, 0, [[1, PW]]),
                        xs[cc][:, h * 96:(h + 1) * 96],
                        wall_sb[cc][:],
                        start=(cc == 0),
                        stop=(cc == 1),
                    )
                qk_dst = _apv(qk[:], g * QKW, [[1, QKW]])
                qk_src = _apv(ps, 0, [[1, QKW]])
                v_dst = _apvp(v, 0, 96, g * 256, [[1, 256]])
                v_src = _apv(ps, QKW, [[1, 256]])
                if g % 2 == 0:
                    nc.vector.tensor_copy(v_dst, v_src)
                    nc.scalar.copy(qk_dst, qk_src)
                else:
                    nc.scalar.copy(v_dst, v_src)
                    nc.vector.tensor_copy(qk_dst, qk_src)

            def stage_ones(b):
                # q channel 32 := 1.0 (pairs with sigma channel of k)
                nc.gpsimd.memset(_apv(st[b]["qk"][:], 32, [[QKW, 96]]), 1.0)

            def stage_transp(b):
                qk, qkt = st[b]["qk"], st[b]["qkt"]
                for grp in range(7):
                    ptq = pj.tile([96, 1024], BF16, tag="ps", name=f"ptq{b}_{grp}")
                    for i in range(min(10, NCHT - grp * 10)):
                        ch = grp * 10 + i
                        nc.tensor.transpose(
                            _apv(ptq, i * 96, [[1, 96]]),
                            _apv(qk[:], ch, [[QKW, 96]]),
                            idtb_sb[:],
                        )
                    nch = min(10, NCHT - grp * 10)
                    dst = _apv(qkt[:], grp * 10, [[NCHT, 96], [1, nch]])
                    srcv = _apv(ptq, 0, [[1, 96], [96, nch]])
                    if grp % 2 == 0:
                        nc.vector.tensor_copy(dst, srcv)
                    else:
                        nc.scalar.copy(dst, srcv)

            def stage_scores_dir(b, d, half):
                # d: 'v' or 'h'; half in (0,1): bands 16*half..16*half+15
                # layout te/A: [p, (3k+j)*32 + band]
                src = st[b]["qkt"] if d == "v" else st[b]["qk"]
                cw = NCHT if d == "v" else QKW
                if half == 0:
                    te = mp.tile([96, 288], BF16, tag=f"te{d}", bufs=1,
                                 name=f"te{d}{b}")
                    A = mp.tile([96, 288], BF16, tag=f"A{d}",
                                bufs=(1 if d == "v" else 2), name=f"A{d}{b}")
                    s3 = mp.tile([96, 96], BF16, tag=f"s3{d}", bufs=1,
                                 name=f"s3{d}{b}")
                    r3 = mp.tile([96, 96], BF16, tag=f"r3{d}", bufs=1,
                                 name=f"r3{d}{b}")
                    bs = mp.tile([96, 96], BF16, tag=f"bs{d}", bufs=1,
                                 name=f"bs{d}{b}")
                    st[b]["te" + d], st[b]["A" + d] = te, A
                    st[b]["s3" + d], st[b]["r3" + d] = s3, r3
                    st[b]["bs" + d] = bs
                te, A = st[b]["te" + d], st[b]["A" + d]
                s3, r3 = st[b]["s3" + d], st[b]["r3" + d]
                bs = st[b]["bs" + d]
                h0 = 16 * half
                for kj in range(9):
                    k, j = divmod(kj, 3)
                    prod = wp.tile([96, 512], BF16, tag="prod")
                    if b == 0:
                        peng = nc.gpsimd
                    else:
                        peng = nc.vector if kj % 3 != 2 else nc.gpsimd
                    peng.tensor_tensor(
                        prod[:, 0:512],
                        _apv(src[:], k * cw + h0 * 3 * cw, [[3 * cw, 16], [1, 32]]),
                        _apv(src[:], j * cw + 33 + h0 * 3 * cw,
                             [[3 * cw, 16], [1, 32]]),
                        MUL,
                    )
                    nc.vector.tensor_reduce(
                        te[:, kj * 32 + h0:kj * 32 + h0 + 16],
                        _apv(prod[:], 0, [[32, 16], [1, 32]]),
                        AXX, ADD,
                    )
                # add sigma (j-pixel bias channel 65) before exp
                nc.vector.tensor_tensor(
                    _apv(te[:], h0, [[96, 3], [32, 3], [1, 16]]),
                    _apv(te[:], h0, [[96, 3], [32, 3], [1, 16]]),
                    _apv(src[:], 65 + h0 * 3 * cw, [[0, 3], [cw, 3], [3 * cw, 16]]),
                    ADD,
                )
                nc.scalar.activation(
                    _apv(te[:], h0, [[32, 9], [1, 16]]),
                    _apv(te[:], h0, [[32, 9], [1, 16]]), EXP)
                nc.vector.tensor_reduce(
                    _apv(s3[:], h0, [[32, 3], [1, 16]]),
                    _apv(te[:], h0, [[96, 3], [1, 16], [32, 3]]),
                    AXX, ADD,
                )
                nc.vector.reciprocal(
                    _apv(r3[:], h0, [[32, 3], [1, 16]]),
                    _apv(s3[:], h0, [[32, 3], [1, 16]]))
                nc.vector.tensor_tensor(
                    _apv(A[:], h0, [[96, 3], [32, 3], [1, 16]]),
                    _apv(te[:], h0, [[96, 3], [32, 3], [1, 16]]),
                    _apv(r3[:], h0, [[32, 3], [0, 3], [1, 16]]),
                    MUL,
                )
                # bias partial sums: sum_k A[k,j] -> bs[p, 3n+j]
                nc.vector.tensor_reduce(
                    _apv(bs[:], 3 * h0, [[3, 16], [1, 3]]),
                    _apv(A[:], h0, [[1, 16], [32, 3], [96, 3]]),
                    AXX, ADD,
                )

            def stage_avt(b):
                # av_p[h, 96j + (3m+k)] = A_v[h, (3k+j)*32+m]
                A = st[b]["Av"]
                av_p = mp.tile([96, 288], BF16, tag="avp2", bufs=1,
                               name=f"avp2{b}")
                nc.vector.tensor_copy(
                    _apv(av_p[:], 0, [[96, 3], [3, 32], [1, 3]]),
                    _apv(A[:], 0, [[32, 3], [1, 32], [96, 3]]),
                )
                avtJ = mp.tile([96, 288], BF16, tag="avtJ", bufs=2,
                               name=f"avtJ{b}")
                for j in range(3):
                    pt = avp.tile([128, 288], BF16, tag="av", name=f"avt{b}_{j}")
                    nc.tensor.transpose(
                        _apvp(pt, 0, 96, 0, [[1, 96]]),
                        av_p[:, j * 96:(j + 1) * 96],
                        idtb_sb[:],
                    )
                    nc.vector.tensor_copy(
                        _apv(avtJ[:], j, [[3, 96]]),
                        _apvp(pt, 0, 96, 0, [[1, 96]]),
                    )
                st[b]["avtJ"] = avtJ

            def stage_btt(b):
                # btT[h, w'] = bsh[w', h]^T + bsv[h, w']
                pt = avp.tile([128, 288], BF16, tag="av", name=f"btt{b}")
                nc.tensor.transpose(
                    _apvp(pt, 0, 96, 0, [[1, 96]]),
                    st[b]["bsh"][:],
                    idtb_sb[:],
                )
                btT = mp.tile([96, 96], RHS_DT, tag="btT", bufs=2, name=f"btT{b}")
                nc.vector.tensor_tensor(
                    btT[:],
                    _apvp(pt, 0, 96, 0, [[1, 96]]),
                    st[b]["bsv"][:],
                    ADD,
                )
                st[b]["btT"] = btT

            def stage_rhs(b, t):
                # rhs tile for bands 4t..4t+3:
                # rhs[w, n*864 + (3k+j)*96 + w'] , row 96 = bias coefs (k=0 slice)
                Ah, avtJ, btT = st[b]["Ah"], st[b]["avtJ"], st[b]["btT"]
                rhs = wp.tile([97, 1728], RHS_DT, tag="rhs", name=f"rhs{b}_{t}")
                eeng = nc.vector if t % 2 == 0 else nc.gpsimd
                eeng.tensor_tensor(
                    _apvp(rhs, 0, 96, 0, [[864, 2], [96, 9], [1, 96]]),
                    _apv(ipat2_sb[:], 0, [[864, 2], [96, 9], [1, 96]]),
                    _apv(Ah[:], 2 * t, [[1, 2], [32, 9], [0, 96]]),
                    MUL,
                )
                for n in range(2):
                    tmp = wp.tile([96, 288], BF16, tag="vtmp")
                    teng = nc.vector if t % 2 == 0 else nc.gpsimd
                    teng.tensor_tensor(
                        _apv(tmp[:], 0, [[96, 3], [3, 32], [1, 3]]),
                        _apv(pstr_sb[:], 0, [[0, 3], [3, 32], [1, 3]]),
                        _apv(avtJ[:], (6 * t + 3 * n) * 3,
                             [[3, 3], [0, 32], [1, 3]]),
                        MUL,
                    )
                    teng.tensor_tensor(
                        _apvp(rhs, 0, 96, n * 864, [[384, 3], [1, 96]]),
                        _apvp(rhs, 0, 96, n * 864, [[384, 3], [1, 96]]),
                        _apv(tmp[:], 0, [[96, 3], [1, 96]]),
                        ADD,
                    )
                # bias row: btT rows -> row 96, k=0 slice per band
                for n in range(2):
                    nc.sync.dma_start(
                        _apvp(rhs, 96, 1, n * 864, [[96, 3], [1, 96]]),
                        btT[6 * t + 3 * n:6 * t + 3 * n + 3, :],
                    )
                return rhs

            def stage_av_tile(b, t, rhs):
                v = st[b]["v"]
                for n in (2 * t, 2 * t + 1):
                    o = (n % 2) * 864
                    psos = []
                    for cc in range(2):
                        psos.append(avp.tile([128, 288], F32, tag="av",
                                             name=f"pso{b}_{n}_{cc}"))
                    for k in range(3):
                        for cc in range(2):
                            np_ = 97 if k == 0 else 96
                            nc.tensor.matmul(
                                psos[cc][:],
                                _apvp(v, 0, np_, (3 * n + k) * 256 + cc * 128,
                                      [[1, 128]]),
                                _apvp(rhs, 0, np_, o + k * 288, [[1, 288]]),
                                start=(k == 0), stop=(k == 2),
                            )
                    for cc in range(2):
                        t8 = st[b][f"t8_{cc}_{n // 8}"]
                        nc.scalar.copy(
                            t8[:, (n % 8) * 288:(n % 8) * 288 + 288],
                            psos[cc][:])

            def stage_tile_fini(b, t):
                # add staged attention of tile t onto xs (2 bands = 576 cols)
                xs = st[b]["xs"]
                grp = (2 * t) // 8
                c0 = (2 * t) % 8 * 288
                for cc in range(2):
                    t8 = st[b][f"t8_{cc}_{grp}"]
                    nc.vector.tensor_tensor(
                        xs[cc][:, 576 * t:576 * t + 576],
                        xs[cc][:, 576 * t:576 * t + 576],
                        t8[:, c0:c0 + 576],
                        ADD,
                    )

            def stage_store(b, half):
                h0 = half * 4608
                for cc in range(2):
                    nc.sync.dma_start(
                        out_e[b, cc, :, h0:h0 + 4608],
                        st[b]["xs"][cc][:, h0:h0 + 4608],
                    )

            def alloc_t8(b):
                for grp in range(4):
                    for cc in range(2):
                        st[b][f"t8_{cc}_{grp}"] = wp.tile(
                            [128, 2304], BF16, tag="t8", bufs=2,
                            name=f"t8_{b}_{cc}_{grp}")

            # ================= emission schedule =================
            stage_load(0)
            for g in range(96):
                stage_proj_group(0, g)
            stage_ones(0)
            stage_transp(0)

            stage_load(1)
            # proj(1) on PE while vector/gpsimd do scores(0)
            for g in range(16):
                stage_proj_group(1, g)
            stage_scores_dir(0, "v", 0)
            for g in range(16, 32):
                stage_proj_group(1, g)
            stage_scores_dir(0, "v", 1)
            for g in range(32, 48):
                stage_proj_group(1, g)
            stage_avt(0)            # PE transposes mid proj(1)
            stage_scores_dir(0, "h", 0)
            for g in range(48, 64):
                stage_proj_group(1, g)
            stage_scores_dir(0, "h", 1)
            for g in range(64, 96):
                stage_proj_group(1, g)
            stage_btt(0)
            stage_ones(1)
            stage_transp(1)

            alloc_t8(0)
            # AV(0) with rhs built one tile ahead; scores(1) interleaved
            rhs_next = stage_rhs(0, 0)
            for t in range(16):
                rhs = rhs_next
                if t < 15:
                    rhs_next = stage_rhs(0, t + 1)
                if t == 0:
                    stage_scores_dir(1, "v", 0)
                if t == 2:
                    stage_scores_dir(1, "v", 1)
                stage_av_tile(0, t, rhs)
                if t > 0:
                    stage_tile_fini(0, t - 1)
                if t == 15:
                    stage_tile_fini(0, 15)
                    stage_store(0, 0)
                    stage_store(0, 1)
                if t == 6:
                    stage_avt(1)
                    stage_scores_dir(1, "h", 0)
                if t == 9:
                    stage_scores_dir(1, "h", 1)
                if t == 12:
                    stage_btt(1)

            alloc_t8(1)
            rhs_next = stage_rhs(1, 0)
            for t in range(16):
                rhs = rhs_next
                if t < 15:
                    rhs_next = stage_rhs(1, t + 1)
                stage_av_tile(1, t, rhs)
                if t > 0:
                    stage_tile_fini(1, t - 1)
                if t == 15:
                    stage_tile_fini(1, 15)
                    stage_store(1, 0)
                    stage_store(1, 1)
    nc.compile()
    return nc


def _host_prep(x, Wq, bq, Wk, bk, Wv, bv, gamma):
    x = np.ascontiguousarray(x, np.float32)
    g = float(np.asarray(gamma).reshape(-1)[0])
    sig_w = (bq @ Wk).astype(np.float32)
    z1 = np.zeros((1, 256), np.float32)
    wall = np.concatenate([Wq, z1, Wk, sig_w[None], Wv * g], 0)  # [322,256]
    wallT = np.stack([np.ascontiguousarray(wall[:, :128].T),
                      np.ascontiguousarray(wall[:, 128:].T)])
    ipat2 = np.tile(np.eye(96), (1, 18)).astype(ml_dtypes.bfloat16)
    pstr = np.kron(np.eye(32), np.ones((3, 3))).astype(ml_dtypes.bfloat16)
    idtb = np.eye(96).astype(ml_dtypes.bfloat16)
    bvrow = np.tile(bv.astype(np.float32) * g, 96)[None, :].astype(
        ml_dtypes.float8_e4m3)
    xr = x.reshape(B, 2, 128, S)
    in_maps = []
    for i in range(NCORE):
        in_maps.append({
            "xa": np.ascontiguousarray(
                xr[i * BPC:(i + 1) * BPC]).astype(ml_dtypes.bfloat16),
            "wall": wallT.astype(ml_dtypes.bfloat16),
            "ipat2": ipat2, "pstr": pstr, "idtb": idtb, "bvrow": bvrow,
        })
    return in_maps


_CACHE = {}


def kernel(x, Wq, bq, Wk, bk, Wv, bv, gamma, _trace=False):
    x = np.asarray(x, np.float32)
    in_maps = _host_prep(x, np.asarray(Wq, np.float32),
                         np.asarray(bq, np.float32),
                         np.asarray(Wk, np.float32),
                         np.asarray(bk, np.float32),
                         np.asarray(Wv, np.float32),
                         np.asarray(bv, np.float32),
                         np.asarray(gamma, np.float32))
    if "nc" not in _CACHE:
        _CACHE["nc"] = build_graph()
    nc = _CACHE["nc"]
    res = run_bass_kernel_spmd(nc, in_maps, list(range(NCORE)), trace=_trace)
    kernel.last_result = res
    out = np.empty((B, C, H, W), np.float32)
    for i in range(NCORE):
        o = np.asarray(res.results[i]["out"], np.float32)
        for b in range(BPC):
            out[i * BPC + b] = o[b].reshape(C, H, W)
    return out


if __name__ == "__main__":
    rng = np.random.default_rng(0)
    xs = {k: rng.standard_normal(s).astype(np.float32) * (0.05 if k != "x" else 1.0)
          for k, s in [("x", (16, 256, 96, 96)), ("Wq", (32, 256)), ("bq", (32,)),
                       ("Wk", (32, 256)), ("bk", (32,)), ("Wv", (256, 256)),
                       ("bv", (256,)), ("gamma", (1,))]}
    y = kernel(**xs)
    print("ran", y.shape)


# revision 4
# speedup vs baseline: 1.0243x; 1.0243x over previous
import sys

for p in ("/opt/trn_rl_repo",):
    if p not in sys.path:
        sys.path.insert(0, p)

import numpy as np
import ml_dtypes

from concourse import bass, mybir, bacc, tile
from concourse.ap import AP
from concourse.bass_utils import run_bass_kernel_spmd


def _install_ntff_hook():
    try:
        from antenv import axon_hooks  # noqa: F401
        return
    except ImportError:
        pass
    import types
    try:
        import antenv
    except ImportError:
        return
    mod = types.ModuleType("antenv.axon_hooks")
    _h = {"hook": None}
    mod.set_axon_ntff_profile_hook = lambda h: _h.__setitem__("hook", h)
    mod.get_axon_ntff_profile_hook = lambda: _h["hook"]
    sys.modules["antenv.axon_hooks"] = mod
    antenv.axon_hooks = mod
    try:
        from trn_agent_boot.trn_boot import _ntff_profile_via_ctypes
        h = _ntff_profile_via_ctypes("/opt/axon/libaxon_pjrt.so")
        if h is not None:
            mod.set_axon_ntff_profile_hook(h)
    except Exception:
        pass


_install_ntff_hook()


def _enable_ldw_opt():
    import concourse.bass_utils as _bu
    if getattr(_bu, "_ldw_patched", False):
        return
    _orig = _bu.run_command

    def _patched(argv, **kw):
        try:
            argv = ["--enable-ldw-opt=true" if c == "--enable-ldw-opt=false" else c
                    for c in argv]
        except TypeError:
            pass
        return _orig(argv, **kw)

    _bu.run_command = _patched
    _bu._ldw_patched = True


# _enable_ldw_opt() breaks walrus codegen (InstLdweights incompatible)

F32 = mybir.dt.float32
BF16 = mybir.dt.bfloat16
F8 = mybir.dt.float8e4
MUL = mybir.AluOpType.mult
ADD = mybir.AluOpType.add
AXX = mybir.AxisListType.X
EXP = mybir.ActivationFunctionType.Exp

B, C, H, W = 16, 256, 96, 96
S = H * W
NB = 32
NCORE = 8
BPC = B // NCORE
QKW = 66            # ch0..31 q, 32 ones, 33..64 k, 65 sigma (no pad)
PW = QKW + 256
NCHT = 66           # transposed channels 0..65

V_DT = F8           # value matrix storage dtype
RHS_DT = F8         # rhs (attention pattern) dtype — must match v dtype for PE


def _apv(t, off, dims):
    b = t[:] if not isinstance(t, AP) else t
    part = list(b.ap[0])
    return AP(b.tensor, b.offset + off, [part] + [list(d) for d in dims])


def _apvp(t, p0, np_, off, dims):
    """View with custom partition range [p0, p0+np_) and custom free dims."""
    b = t[:] if not isinstance(t, AP) else t
    part = [b.ap[0][0], np_]
    return AP(b.tensor, b.offset + p0 * b.ap[0][0] + off,
              [part] + [list(d) for d in dims])


def build_graph():
    nc = bacc.Bacc(None, target_bir_lowering=False)

    xa_e = nc.declare_dram_parameter("xa", [BPC, 2, 128, S], BF16, isOutput=False)
    wall_e = nc.declare_dram_parameter("wall", [2, 128, PW], BF16, isOutput=False)
    ipat2_e = nc.declare_dram_parameter("ipat2", [96, 1728], BF16, isOutput=False)
    pstr_e = nc.declare_dram_parameter("pstr", [96, 96], BF16, isOutput=False)
    idtb_e = nc.declare_dram_parameter("idtb", [96, 96], BF16, isOutput=False)
    bvrow_e = nc.declare_dram_parameter("bvrow", [1, 96 * 256], V_DT, isOutput=False)
    out_e = nc.declare_dram_parameter("out", [BPC, 2, 128, S], BF16, isOutput=True)

    with tile.TileContext(nc) as tc, nc.allow_low_precision(reason="bf16 score accum ok for 2e-2 tol"):
        with (
            tc.tile_pool(name="const", bufs=1) as cp,
            tc.tile_pool(name="main", bufs=1) as mp,
            tc.tile_pool(name="work", bufs=2) as wp,
            tc.tile_pool(name="pj", bufs=4, space="PSUM") as pj,
            tc.tile_pool(name="avp", bufs=4, space="PSUM") as avp,
        ):
            wall_sb = []
            for cc in range(2):
                t = cp.tile([128, PW], BF16, tag=f"wall{cc}")
                nc.sync.dma_start(t[:], wall_e[cc])
                wall_sb.append(t)
            ipat2_sb = cp.tile([96, 1728], BF16, tag="ipat2")
            nc.sync.dma_start(ipat2_sb[:], ipat2_e[:])
            pstr_sb = cp.tile([96, 96], BF16, tag="pstr")
            nc.sync.dma_start(pstr_sb[:], pstr_e[:])
            idtb_sb = cp.tile([96, 96], BF16, tag="idtb")
            nc.sync.dma_start(idtb_sb[:], idtb_e[:])

            st = {}

            def stage_load(b):
                xs = []
                for cc in range(2):
                    t = mp.tile([128, S], BF16, tag=f"xa{cc}", bufs=2,
                                name=f"xa{cc}_{b}")
                    xs.append(t)
                # interleave cc chunks so early proj groups unblock fast
                for q in range(4):
                    for cc in range(2):
                        nc.sync.dma_start(xs[cc][:, q * 2304:(q + 1) * 2304],
                                          xa_e[b, cc, :, q * 2304:(q + 1) * 2304])
                v = mp.tile([97, 96 * 256], V_DT, tag="v", bufs=2, name=f"v{b}")
                nc.sync.dma_start(v[96:97, :], bvrow_e[:])
                qk = mp.tile([96, 96 * QKW], BF16, tag="qk", bufs=2, name=f"qk{b}")
                qkt = mp.tile([96, 96 * NCHT], BF16, tag="qkt", bufs=2,
                              name=f"qkt{b}")
                st[b] = {"xs": xs, "v": v, "qk": qk, "qkt": qkt}

            def stage_proj_group(b, g):
                # 1 line per group, g in 0..95
                xs, v, qk = st[b]["xs"], st[b]["v"], st[b]["qk"]
                ps = pj.tile([96, 512], F32, tag="ps", name=f"ps{b}_{g}")
                h = g
                for cc in range(2):
                    nc.tensor.matmul(
                        _apv(ps, 0, [[1, PW]]),
                        xs[cc][:, h * 96:(h + 1) * 96],
                        wall_sb[cc][:],
                        start=(cc == 0),
                        stop=(cc == 1),
                    )
                qk_dst = _apv(qk[:], g * QKW, [[1, QKW]])
                qk_src = _apv(ps, 0, [[1, QKW]])
                v_dst = _apvp(v, 0, 96, g * 256, [[1, 256]])
                v_src = _apv(ps, QKW, [[1, 256]])
                if g % 2 == 0:
                    nc.vector.tensor_copy(v_dst, v_src)
                    nc.scalar.copy(qk_dst, qk_src)
                else:
                    nc.scalar.copy(v_dst, v_src)
                    nc.vector.tensor_copy(qk_dst, qk_src)

            def stage_ones(b):
                # q channel 32 := 1.0 (pairs with sigma channel of k)
                nc.gpsimd.memset(_apv(st[b]["qk"][:], 32, [[QKW, 96]]), 1.0)

            def stage_transp(b):
                qk, qkt = st[b]["qk"], st[b]["qkt"]
                for grp in range(7):
                    ptq = pj.tile([96, 1024], BF16, tag="ps", name=f"ptq{b}_{grp}")
                    for i in range(min(10, NCHT - grp * 10)):
                        ch = grp * 10 + i
                        nc.tensor.transpose(
                            _apv(ptq, i * 96, [[1, 96]]),
                            _apv(qk[:], ch, [[QKW, 96]]),
                            idtb_sb[:],
                        )
                    nch = min(10, NCHT - grp * 10)
                    dst = _apv(qkt[:], grp * 10, [[NCHT, 96], [1, nch]])
                    srcv = _apv(ptq, 0, [[1, 96], [96, nch]])
                    if grp % 2 == 0:
                        nc.vector.tensor_copy(dst, srcv)
                    else:
                        nc.scalar.copy(dst, srcv)

            def stage_scores_dir(b, d, half):
                # d: 'v' or 'h'; half in (0,1): bands 16*half..16*half+15
                # layout te/A: [p, (3k+j)*32 + band]
                src = st[b]["qkt"] if d == "v" else st[b]["qk"]
                cw = NCHT if d == "v" else QKW
                if half == 0:
                    te = mp.tile([96, 288], BF16, tag=f"te{d}", bufs=1,
                                 name=f"te{d}{b}")
                    A = mp.tile([96, 288], BF16, tag=f"A{d}",
                                bufs=(1 if d == "v" else 2), name=f"A{d}{b}")
                    s3 = mp.tile([96, 96], BF16, tag=f"s3{d}", bufs=1,
                                 name=f"s3{d}{b}")
                    r3 = mp.tile([96, 96], BF16, tag=f"r3{d}", bufs=1,
                                 name=f"r3{d}{b}")
                    bs = mp.tile([96, 96], BF16, tag=f"bs{d}", bufs=1,
                                 name=f"bs{d}{b}")
                    st[b]["te" + d], st[b]["A" + d] = te, A
                    st[b]["s3" + d], st[b]["r3" + d] = s3, r3
                    st[b]["bs" + d] = bs
                te, A = st[b]["te" + d], st[b]["A" + d]
                s3, r3 = st[b]["s3" + d], st[b]["r3" + d]
                bs = st[b]["bs" + d]
                h0 = 16 * half
                for kj in range(9):
                    k, j = divmod(kj, 3)
                    prod = wp.tile([96, 512], BF16, tag="prod")
                    if b == 0:
                        peng = nc.gpsimd
                    else:
                        peng = nc.vector if kj % 3 != 2 else nc.gpsimd
                    peng.tensor_tensor(
                        prod[:, 0:512],
                        _apv(src[:], k * cw + h0 * 3 * cw, [[3 * cw, 16], [1, 32]]),
                        _apv(src[:], j * cw + 33 + h0 * 3 * cw,
                             [[3 * cw, 16], [1, 32]]),
                        MUL,
                    )
                    nc.vector.tensor_reduce(
                        te[:, kj * 32 + h0:kj * 32 + h0 + 16],
                        _apv(prod[:], 0, [[32, 16], [1, 32]]),
                        AXX, ADD,
                    )
                # add sigma (j-pixel bias channel 65) before exp
                nc.vector.tensor_tensor(
                    _apv(te[:], h0, [[96, 3], [32, 3], [1, 16]]),
                    _apv(te[:], h0, [[96, 3], [32, 3], [1, 16]]),
                    _apv(src[:], 65 + h0 * 3 * cw, [[0, 3], [cw, 3], [3 * cw, 16]]),
                    ADD,
                )
                nc.scalar.activation(
                    _apv(te[:], h0, [[32, 9], [1, 16]]),
                    _apv(te[:], h0, [[32, 9], [1, 16]]), EXP)
                nc.vector.tensor_reduce(
                    _apv(s3[:], h0, [[32, 3], [1, 16]]),
                    _apv(te[:], h0, [[96, 3], [1, 16], [32, 3]]),
                    AXX, ADD,
                )
                nc.vector.reciprocal(
                    _apv(r3[:], h0, [[32, 3], [1, 16]]),
                    _apv(s3[:], h0, [[32, 3], [1, 16]]))
                nc.vector.tensor_tensor(
                    _apv(A[:], h0, [[96, 3], [32, 3], [1, 16]]),
                    _apv(te[:], h0, [[96, 3], [32, 3], [1, 16]]),
                    _apv(r3[:], h0, [[32, 3], [0, 3], [1, 16]]),
                    MUL,
                )
                # bias partial sums: sum_k A[k,j] -> bs[p, 3n+j]
                nc.vector.tensor_reduce(
                    _apv(bs[:], 3 * h0, [[3, 16], [1, 3]]),
                    _apv(A[:], h0, [[1, 16], [32, 3], [96, 3]]),
                    AXX, ADD,
                )

            def stage_avt(b):
                # av_p[h, 96j + (3m+k)] = A_v[h, (3k+j)*32+m]
                A = st[b]["Av"]
                av_p = mp.tile([96, 288], BF16, tag="avp2", bufs=1,
                               name=f"avp2{b}")
                nc.vector.tensor_copy(
                    _apv(av_p[:], 0, [[96, 3], [3, 32], [1, 3]]),
                    _apv(A[:], 0, [[32, 3], [1, 32], [96, 3]]),
                )
                avtJ = mp.tile([96, 288], BF16, tag="avtJ", bufs=2,
                               name=f"avtJ{b}")
                for j in range(3):
                    pt = avp.tile([128, 288], BF16, tag="av", name=f"avt{b}_{j}")
                    nc.tensor.transpose(
                        _apvp(pt, 0, 96, 0, [[1, 96]]),
                        av_p[:, j * 96:(j + 1) * 96],
                        idtb_sb[:],
                    )
                    nc.vector.tensor_copy(
                        _apv(avtJ[:], j, [[3, 96]]),
                        _apvp(pt, 0, 96, 0, [[1, 96]]),
                    )
                st[b]["avtJ"] = avtJ

            def stage_btt(b):
                # btT[h, w'] = bsh[w', h]^T + bsv[h, w']
                pt = avp.tile([128, 288], BF16, tag="av", name=f"btt{b}")
                nc.tensor.transpose(
                    _apvp(pt, 0, 96, 0, [[1, 96]]),
                    st[b]["bsh"][:],
                    idtb_sb[:],
                )
                btT = mp.tile([96, 96], RHS_DT, tag="btT", bufs=2, name=f"btT{b}")
                nc.vector.tensor_tensor(
                    btT[:],
                    _apvp(pt, 0, 96, 0, [[1, 96]]),
                    st[b]["bsv"][:],
                    ADD,
                )
                st[b]["btT"] = btT

            def stage_rhs(b, t):
                # rhs tile for bands 4t..4t+3:
                # rhs[w, n*864 + (3k+j)*96 + w'] , row 96 = bias coefs (k=0 slice)
                Ah, avtJ, btT = st[b]["Ah"], st[b]["avtJ"], st[b]["btT"]
                rhs = wp.tile([97, 1728], RHS_DT, tag="rhs", bufs=3, name=f"rhs{b}_{t}")
                eeng = nc.vector if t % 2 == 0 else nc.gpsimd
                eeng.tensor_tensor(
                    _apvp(rhs, 0, 96, 0, [[864, 2], [96, 9], [1, 96]]),
                    _apv(ipat2_sb[:], 0, [[864, 2], [96, 9], [1, 96]]),
                    _apv(Ah[:], 2 * t, [[1, 2], [32, 9], [0, 96]]),
                    MUL,
                )
                for n in range(2):
                    tmp = wp.tile([96, 288], BF16, tag="vtmp")
                    teng = nc.vector if t % 2 == 0 else nc.gpsimd
                    teng.tensor_tensor(
                        _apv(tmp[:], 0, [[96, 3], [3, 32], [1, 3]]),
                        _apv(pstr_sb[:], 0, [[0, 3], [3, 32], [1, 3]]),
                        _apv(avtJ[:], (6 * t + 3 * n) * 3,
                             [[3, 3], [0, 32], [1, 3]]),
                        MUL,
                    )
                    teng.tensor_tensor(
                        _apvp(rhs, 0, 96, n * 864, [[384, 3], [1, 96]]),
                        _apvp(rhs, 0, 96, n * 864, [[384, 3], [1, 96]]),
                        _apv(tmp[:], 0, [[96, 3], [1, 96]]),
                        ADD,
                    )
                # bias row: btT rows -> row 96, k=0 slice per band
                for n in range(2):
                    nc.sync.dma_start(
                        _apvp(rhs, 96, 1, n * 864, [[96, 3], [1, 96]]),
                        btT[6 * t + 3 * n:6 * t + 3 * n + 3, :],
                    )
                return rhs

            def stage_av_tile(b, t, rhs):
                v = st[b]["v"]
                for n in (2 * t, 2 * t + 1):
                    o = (n % 2) * 864
                    psos = []
                    for cc in range(2):
                        psos.append(avp.tile([128, 288], F32, tag="av",
                                             name=f"pso{b}_{n}_{cc}"))
                    for k in range(3):
                        for cc in range(2):
                            np_ = 97 if k == 0 else 96
                            nc.tensor.matmul(
                                psos[cc][:],
                                _apvp(v, 0, np_, (3 * n + k) * 256 + cc * 128,
                                      [[1, 128]]),
                                _apvp(rhs, 0, np_, o + k * 288, [[1, 288]]),
                                start=(k == 0), stop=(k == 2),
                            )
                    for cc in range(2):
                        t8 = st[b][f"t8_{cc}_{n // 8}"]
                        nc.scalar.copy(
                            t8[:, (n % 8) * 288:(n % 8) * 288 + 288],
                            psos[cc][:])

            def stage_tile_fini(b, t):
                # add staged attention of tile t onto xs (2 bands = 576 cols)
                xs = st[b]["xs"]
                grp = (2 * t) // 8
                c0 = (2 * t) % 8 * 288
                for cc in range(2):
                    t8 = st[b][f"t8_{cc}_{grp}"]
                    nc.vector.tensor_tensor(
                        xs[cc][:, 576 * t:576 * t + 576],
                        xs[cc][:, 576 * t:576 * t + 576],
                        t8[:, c0:c0 + 576],
                        ADD,
                    )

            def stage_store(b, half):
                h0 = half * 4608
                for cc in range(2):
                    nc.sync.dma_start(
                        out_e[b, cc, :, h0:h0 + 4608],
                        st[b]["xs"][cc][:, h0:h0 + 4608],
                    )

            def alloc_t8(b):
                for grp in range(4):
                    for cc in range(2):
                        st[b][f"t8_{cc}_{grp}"] = wp.tile(
                            [128, 2304], BF16, tag="t8", bufs=2,
                            name=f"t8_{b}_{cc}_{grp}")

            # ================= emission schedule =================
            stage_load(0)
            for g in range(96):
                stage_proj_group(0, g)
            stage_ones(0)
            stage_transp(0)

            stage_load(1)
            # proj(1) on PE while vector/gpsimd do scores(0)
            for g in range(16):
                stage_proj_group(1, g)
            stage_scores_dir(0, "v", 0)
            for g in range(16, 32):
                stage_proj_group(1, g)
            stage_scores_dir(0, "v", 1)
            for g in range(32, 48):
                stage_proj_group(1, g)
            stage_avt(0)            # PE transposes mid proj(1)
            stage_scores_dir(0, "h", 0)
            for g in range(48, 64):
                stage_proj_group(1, g)
            stage_scores_dir(0, "h", 1)
            for g in range(64, 96):
                stage_proj_group(1, g)
            stage_btt(0)
            stage_ones(1)
            stage_transp(1)

            alloc_t8(0)
            # AV(0) with rhs built one tile ahead; scores(1) interleaved
            rhs_q = [stage_rhs(0, 0), stage_rhs(0, 1)]
            for t in range(16):
                rhs = rhs_q.pop(0)
                if t < 14:
                    rhs_q.append(stage_rhs(0, t + 2))
                if t == 0:
                    stage_scores_dir(1, "v", 0)
                if t == 2:
                    stage_scores_dir(1, "v", 1)
                stage_av_tile(0, t, rhs)
                if t > 0:
                    stage_tile_fini(0, t - 1)
                if t == 15:
                    stage_tile_fini(0, 15)
                    stage_store(0, 0)
                    stage_store(0, 1)
                if t == 6:
                    stage_avt(1)
                    stage_scores_dir(1, "h", 0)
                if t == 9:
                    stage_scores_dir(1, "h", 1)
                if t == 12:
                    stage_btt(1)

            alloc_t8(1)
            rhs_q = [stage_rhs(1, 0), stage_rhs(1, 1)]
            for t in range(16):
                rhs = rhs_q.pop(0)
                if t < 14:
                    rhs_q.append(stage_rhs(1, t + 2))
                stage_av_tile(1, t, rhs)
                if t > 0:
                    stage_tile_fini(1, t - 1)
                if t == 15:
                    stage_tile_fini(1, 15)
                    stage_store(1, 0)
                    stage_store(1, 1)
    nc.compile()
    return nc


def _host_prep(x, Wq, bq, Wk, bk, Wv, bv, gamma):
    x = np.ascontiguousarray(x, np.float32)
    g = float(np.asarray(gamma).reshape(-1)[0])
    sig_w = (bq @ Wk).astype(np.float32)
    z1 = np.zeros((1, 256), np.float32)
    wall = np.concatenate([Wq, z1, Wk, sig_w[None], Wv * g], 0)  # [322,256]
    wallT = np.stack([np.ascontiguousarray(wall[:, :128].T),
                      np.ascontiguousarray(wall[:, 128:].T)])
    ipat2 = np.tile(np.eye(96), (1, 18)).astype(ml_dtypes.bfloat16)
    pstr = np.kron(np.eye(32), np.ones((3, 3))).astype(ml_dtypes.bfloat16)
    idtb = np.eye(96).astype(ml_dtypes.bfloat16)
    bvrow = np.tile(bv.astype(np.float32) * g, 96)[None, :].astype(
        ml_dtypes.float8_e4m3)
    xr = x.reshape(B, 2, 128, S)
    in_maps = []
    for i in range(NCORE):
        in_maps.append({
            "xa": np.ascontiguousarray(
                xr[i * BPC:(i + 1) * BPC]).astype(ml_dtypes.bfloat16),
            "wall": wallT.astype(ml_dtypes.bfloat16),
            "ipat2": ipat2, "pstr": pstr, "idtb": idtb, "bvrow": bvrow,
        })
    return in_maps


_CACHE = {}


def kernel(x, Wq, bq, Wk, bk, Wv, bv, gamma, _trace=False):
    x = np.asarray(x, np.float32)
    in_maps = _host_prep(x, np.asarray(Wq, np.float32),
                         np.asarray(bq, np.float32),
                         np.asarray(Wk, np.float32),
                         np.asarray(bk, np.float32),
                         np.asarray(Wv, np.float32),
                         np.asarray(bv, np.float32),
                         np.asarray(gamma, np.float32))
    if "nc" not in _CACHE:
        _CACHE["nc"] = build_graph()
    nc = _CACHE["nc"]
    res = run_bass_kernel_spmd(nc, in_maps, list(range(NCORE)), trace=_trace)
    kernel.last_result = res
    out = np.empty((B, C, H, W), np.float32)
    for i in range(NCORE):
        o = np.asarray(res.results[i]["out"], np.float32)
        for b in range(BPC):
            out[i * BPC + b] = o[b].reshape(C, H, W)
    return out


if __name__ == "__main__":
    rng = np.random.default_rng(0)
    xs = {k: rng.standard_normal(s).astype(np.float32) * (0.05 if k != "x" else 1.0)
          for k, s in [("x", (16, 256, 96, 96)), ("Wq", (32, 256)), ("bq", (32,)),
                       ("Wk", (32, 256)), ("bk", (32,)), ("Wv", (256, 256)),
                       ("bv", (256,)), ("gamma", (1,))]}
    y = kernel(**xs)
    print("ran", y.shape)


# revision 5
# speedup vs baseline: 1.0712x; 1.0458x over previous
import sys

for p in ("/opt/trn_rl_repo",):
    if p not in sys.path:
        sys.path.insert(0, p)

import numpy as np
import ml_dtypes

from concourse import bass, mybir, bacc, tile
from concourse.ap import AP
from concourse.bass_utils import run_bass_kernel_spmd


def _install_ntff_hook():
    try:
        from antenv import axon_hooks  # noqa: F401
        return
    except ImportError:
        pass
    import types
    try:
        import antenv
    except ImportError:
        return
    mod = types.ModuleType("antenv.axon_hooks")
    _h = {"hook": None}
    mod.set_axon_ntff_profile_hook = lambda h: _h.__setitem__("hook", h)
    mod.get_axon_ntff_profile_hook = lambda: _h["hook"]
    sys.modules["antenv.axon_hooks"] = mod
    antenv.axon_hooks = mod
    try:
        from trn_agent_boot.trn_boot import _ntff_profile_via_ctypes
        h = _ntff_profile_via_ctypes("/opt/axon/libaxon_pjrt.so")
        if h is not None:
            mod.set_axon_ntff_profile_hook(h)
    except Exception:
        pass


_install_ntff_hook()


def _enable_ldw_opt():
    import concourse.bass_utils as _bu
    if getattr(_bu, "_ldw_patched", False):
        return
    _orig = _bu.run_command

    def _patched(argv, **kw):
        try:
            argv = ["--enable-ldw-opt=true" if c == "--enable-ldw-opt=false" else c
                    for c in argv]
        except TypeError:
            pass
        return _orig(argv, **kw)

    _bu.run_command = _patched
    _bu._ldw_patched = True


# _enable_ldw_opt() breaks walrus codegen (InstLdweights incompatible)

F32 = mybir.dt.float32
BF16 = mybir.dt.bfloat16
F8 = mybir.dt.float8e4
MUL = mybir.AluOpType.mult
ADD = mybir.AluOpType.add
AXX = mybir.AxisListType.X
EXP = mybir.ActivationFunctionType.Exp

B, C, H, W = 16, 256, 96, 96
S = H * W
NB = 32
NCORE = 8
BPC = B // NCORE
QKW = 66            # ch0..31 q, 32 ones, 33..64 k, 65 sigma (no pad)
PW = QKW + 256
NCHT = 66           # transposed channels 0..65

V_DT = F8           # value matrix storage dtype
RHS_DT = F8         # rhs (attention pattern) dtype — must match v dtype for PE


def _apv(t, off, dims):
    b = t[:] if not isinstance(t, AP) else t
    part = list(b.ap[0])
    return AP(b.tensor, b.offset + off, [part] + [list(d) for d in dims])


def _apvp(t, p0, np_, off, dims):
    """View with custom partition range [p0, p0+np_) and custom free dims."""
    b = t[:] if not isinstance(t, AP) else t
    part = [b.ap[0][0], np_]
    return AP(b.tensor, b.offset + p0 * b.ap[0][0] + off,
              [part] + [list(d) for d in dims])


def build_graph():
    nc = bacc.Bacc(None, target_bir_lowering=False)

    xa_e = nc.declare_dram_parameter("xa", [BPC, 2, 128, S], BF16, isOutput=False)
    wall_e = nc.declare_dram_parameter("wall", [2, 128, PW], BF16, isOutput=False)
    ipat2_e = nc.declare_dram_parameter("ipat2", [96, 1728], BF16, isOutput=False)
    pstr_e = nc.declare_dram_parameter("pstr", [96, 96], BF16, isOutput=False)
    idtb_e = nc.declare_dram_parameter("idtb", [96, 96], BF16, isOutput=False)
    bvrow_e = nc.declare_dram_parameter("bvrow", [1, 96 * 256], V_DT, isOutput=False)
    out_e = nc.declare_dram_parameter("out", [BPC, 2, 128, S], BF16, isOutput=True)

    with tile.TileContext(nc) as tc, nc.allow_low_precision(reason="bf16 score accum ok for 2e-2 tol"):
        with (
            tc.tile_pool(name="const", bufs=1) as cp,
            tc.tile_pool(name="main", bufs=1) as mp,
            tc.tile_pool(name="work", bufs=2) as wp,
            tc.tile_pool(name="pj", bufs=5, space="PSUM") as pj,
            tc.tile_pool(name="avp", bufs=3, space="PSUM") as avp,
        ):
            wall_sb = []
            for cc in range(2):
                t = cp.tile([128, PW], BF16, tag=f"wall{cc}")
                nc.sync.dma_start(t[:], wall_e[cc])
                wall_sb.append(t)
            ipat2_sb = cp.tile([96, 1728], BF16, tag="ipat2")
            nc.sync.dma_start(ipat2_sb[:], ipat2_e[:])
            pstr_sb = cp.tile([96, 96], BF16, tag="pstr")
            nc.sync.dma_start(pstr_sb[:], pstr_e[:])
            idtb_sb = cp.tile([96, 96], BF16, tag="idtb")
            nc.sync.dma_start(idtb_sb[:], idtb_e[:])

            st = {}

            def stage_load(b):
                xs = []
                for cc in range(2):
                    t = mp.tile([128, S], BF16, tag=f"xa{cc}", bufs=2,
                                name=f"xa{cc}_{b}")
                    xs.append(t)
                # interleave cc chunks so early proj groups unblock fast
                for q in range(4):
                    for cc in range(2):
                        nc.sync.dma_start(xs[cc][:, q * 2304:(q + 1) * 2304],
                                          xa_e[b, cc, :, q * 2304:(q + 1) * 2304])
                v = mp.tile([97, 96 * 256], V_DT, tag="v", bufs=2, name=f"v{b}")
                nc.sync.dma_start(v[96:97, :], bvrow_e[:])
                qk = mp.tile([96, 96 * QKW], BF16, tag="qk", bufs=2, name=f"qk{b}")
                qkt = mp.tile([96, 96 * NCHT], BF16, tag="qkt", bufs=2,
                              name=f"qkt{b}")
                st[b] = {"xs": xs, "v": v, "qk": qk, "qkt": qkt}

            def stage_proj_group(b, g):
                # 1 line per group, g in 0..95
                xs, v, qk = st[b]["xs"], st[b]["v"], st[b]["qk"]
                ps = pj.tile([96, 512], F32, tag="ps", name=f"ps{b}_{g}")
                h = g
                for cc in range(2):
                    nc.tensor.matmul(
                        _apv(ps, 0, [[1, PW]]),
                        xs[cc][:, h * 96:(h + 1) * 96],
                        wall_sb[cc][:],
                        start=(cc == 0),
                        stop=(cc == 1),
                    )
                qk_dst = _apv(qk[:], g * QKW, [[1, QKW]])
                qk_src = _apv(ps, 0, [[1, QKW]])
                v_dst = _apvp(v, 0, 96, g * 256, [[1, 256]])
                v_src = _apv(ps, QKW, [[1, 256]])
                if g % 2 == 0:
                    nc.vector.tensor_copy(v_dst, v_src)
                    nc.scalar.copy(qk_dst, qk_src)
                else:
                    nc.scalar.copy(v_dst, v_src)
                    nc.vector.tensor_copy(qk_dst, qk_src)

            def stage_ones(b):
                # q channel 32 := 1.0 (pairs with sigma channel of k)
                nc.gpsimd.memset(_apv(st[b]["qk"][:], 32, [[QKW, 96]]), 1.0)

            def stage_transp(b):
                qk, qkt = st[b]["qk"], st[b]["qkt"]
                for grp in range(7):
                    ptq = pj.tile([96, 1024], BF16, tag="ps", name=f"ptq{b}_{grp}")
                    for i in range(min(10, NCHT - grp * 10)):
                        ch = grp * 10 + i
                        nc.tensor.transpose(
                            _apv(ptq, i * 96, [[1, 96]]),
                            _apv(qk[:], ch, [[QKW, 96]]),
                            idtb_sb[:],
                        )
                    nch = min(10, NCHT - grp * 10)
                    dst = _apv(qkt[:], grp * 10, [[NCHT, 96], [1, nch]])
                    srcv = _apv(ptq, 0, [[1, 96], [96, nch]])
                    if grp % 2 == 0:
                        nc.vector.tensor_copy(dst, srcv)
                    else:
                        nc.scalar.copy(dst, srcv)

            def stage_scores_dir(b, d, half):
                # d: 'v' or 'h'; half in (0,1): bands 16*half..16*half+15
                # layout te/A: [p, (3k+j)*32 + band]
                src = st[b]["qkt"] if d == "v" else st[b]["qk"]
                cw = NCHT if d == "v" else QKW
                if half == 0:
                    te = mp.tile([96, 288], BF16, tag=f"te{d}", bufs=1,
                                 name=f"te{d}{b}")
                    A = mp.tile([96, 288], BF16, tag=f"A{d}",
                                bufs=(1 if d == "v" else 2), name=f"A{d}{b}")
                    s3 = mp.tile([96, 96], BF16, tag=f"s3{d}", bufs=1,
                                 name=f"s3{d}{b}")
                    r3 = mp.tile([96, 96], BF16, tag=f"r3{d}", bufs=1,
                                 name=f"r3{d}{b}")
                    bs = mp.tile([96, 96], BF16, tag=f"bs{d}", bufs=1,
                                 name=f"bs{d}{b}")
                    st[b]["te" + d], st[b]["A" + d] = te, A
                    st[b]["s3" + d], st[b]["r3" + d] = s3, r3
                    st[b]["bs" + d] = bs
                te, A = st[b]["te" + d], st[b]["A" + d]
                s3, r3 = st[b]["s3" + d], st[b]["r3" + d]
                bs = st[b]["bs" + d]
                h0 = 16 * half
                for kj in range(9):
                    k, j = divmod(kj, 3)
                    prod = wp.tile([96, 512], BF16, tag="prod")
                    if b == 0:
                        peng = nc.gpsimd
                    else:
                        peng = nc.vector if kj % 3 != 2 else nc.gpsimd
                    peng.tensor_tensor(
                        prod[:, 0:512],
                        _apv(src[:], k * cw + h0 * 3 * cw, [[3 * cw, 16], [1, 32]]),
                        _apv(src[:], j * cw + 33 + h0 * 3 * cw,
                             [[3 * cw, 16], [1, 32]]),
                        MUL,
                    )
                    nc.vector.tensor_reduce(
                        te[:, kj * 32 + h0:kj * 32 + h0 + 16],
                        _apv(prod[:], 0, [[32, 16], [1, 32]]),
                        AXX, ADD,
                    )
                # add sigma (j-pixel bias channel 65) before exp
                nc.vector.tensor_tensor(
                    _apv(te[:], h0, [[96, 3], [32, 3], [1, 16]]),
                    _apv(te[:], h0, [[96, 3], [32, 3], [1, 16]]),
                    _apv(src[:], 65 + h0 * 3 * cw, [[0, 3], [cw, 3], [3 * cw, 16]]),
                    ADD,
                )
                nc.scalar.activation(
                    _apv(te[:], h0, [[32, 9], [1, 16]]),
                    _apv(te[:], h0, [[32, 9], [1, 16]]), EXP)
                nc.vector.tensor_reduce(
                    _apv(s3[:], h0, [[32, 3], [1, 16]]),
                    _apv(te[:], h0, [[96, 3], [1, 16], [32, 3]]),
                    AXX, ADD,
                )
                nc.vector.reciprocal(
                    _apv(r3[:], h0, [[32, 3], [1, 16]]),
                    _apv(s3[:], h0, [[32, 3], [1, 16]]))
                nc.vector.tensor_tensor(
                    _apv(A[:], h0, [[96, 3], [32, 3], [1, 16]]),
                    _apv(te[:], h0, [[96, 3], [32, 3], [1, 16]]),
                    _apv(r3[:], h0, [[32, 3], [0, 3], [1, 16]]),
                    MUL,
                )
                # bias partial sums: sum_k A[k,j] -> bs[p, 3n+j]
                nc.vector.tensor_reduce(
                    _apv(bs[:], 3 * h0, [[3, 16], [1, 3]]),
                    _apv(A[:], h0, [[1, 16], [32, 3], [96, 3]]),
                    AXX, ADD,
                )

            def stage_avt(b):
                # av_p[h, 96j + (3m+k)] = A_v[h, (3k+j)*32+m]
                A = st[b]["Av"]
                av_p = mp.tile([96, 288], BF16, tag="avp2", bufs=1,
                               name=f"avp2{b}")
                nc.vector.tensor_copy(
                    _apv(av_p[:], 0, [[96, 3], [3, 32], [1, 3]]),
                    _apv(A[:], 0, [[32, 3], [1, 32], [96, 3]]),
                )
                avtJ = mp.tile([96, 288], BF16, tag="avtJ", bufs=2,
                               name=f"avtJ{b}")
                for j in range(3):
                    pt = avp.tile([128, 288], BF16, tag="av", name=f"avt{b}_{j}")
                    nc.tensor.transpose(
                        _apvp(pt, 0, 96, 0, [[1, 96]]),
                        av_p[:, j * 96:(j + 1) * 96],
                        idtb_sb[:],
                    )
                    nc.vector.tensor_copy(
                        _apv(avtJ[:], j, [[3, 96]]),
                        _apvp(pt, 0, 96, 0, [[1, 96]]),
                    )
                st[b]["avtJ"] = avtJ

            def stage_btt(b):
                # btT[h, w'] = bsh[w', h]^T + bsv[h, w']
                pt = avp.tile([128, 288], BF16, tag="av", name=f"btt{b}")
                nc.tensor.transpose(
                    _apvp(pt, 0, 96, 0, [[1, 96]]),
                    st[b]["bsh"][:],
                    idtb_sb[:],
                )
                btT = mp.tile([96, 96], RHS_DT, tag="btT", bufs=2, name=f"btT{b}")
                nc.vector.tensor_tensor(
                    btT[:],
                    _apvp(pt, 0, 96, 0, [[1, 96]]),
                    st[b]["bsv"][:],
                    ADD,
                )
                st[b]["btT"] = btT

            def stage_rhs(b, t):
                # rhs tile for bands 4t..4t+3:
                # rhs[w, n*864 + (3k+j)*96 + w'] , row 96 = bias coefs (k=0 slice)
                Ah, avtJ, btT = st[b]["Ah"], st[b]["avtJ"], st[b]["btT"]
                rhs = wp.tile([97, 1728], RHS_DT, tag="rhs", bufs=3, name=f"rhs{b}_{t}")
                eeng = nc.vector if t % 2 == 0 else nc.gpsimd
                eeng.tensor_tensor(
                    _apvp(rhs, 0, 96, 0, [[864, 2], [96, 9], [1, 96]]),
                    _apv(ipat2_sb[:], 0, [[864, 2], [96, 9], [1, 96]]),
                    _apv(Ah[:], 2 * t, [[1, 2], [32, 9], [0, 96]]),
                    MUL,
                )
                for n in range(2):
                    tmp = wp.tile([96, 288], BF16, tag="vtmp")
                    teng = nc.vector if t % 2 == 0 else nc.gpsimd
                    teng.tensor_tensor(
                        _apv(tmp[:], 0, [[96, 3], [3, 32], [1, 3]]),
                        _apv(pstr_sb[:], 0, [[0, 3], [3, 32], [1, 3]]),
                        _apv(avtJ[:], (6 * t + 3 * n) * 3,
                             [[3, 3], [0, 32], [1, 3]]),
                        MUL,
                    )
                    teng.tensor_tensor(
                        _apvp(rhs, 0, 96, n * 864, [[384, 3], [1, 96]]),
                        _apvp(rhs, 0, 96, n * 864, [[384, 3], [1, 96]]),
                        _apv(tmp[:], 0, [[96, 3], [1, 96]]),
                        ADD,
                    )
                # bias row: btT rows -> row 96, k=0 slice per band
                for n in range(2):
                    nc.sync.dma_start(
                        _apvp(rhs, 96, 1, n * 864, [[96, 3], [1, 96]]),
                        btT[6 * t + 3 * n:6 * t + 3 * n + 3, :],
                    )
                return rhs

            def stage_av_tile(b, t, rhs):
                v = st[b]["v"]
                for n in (2 * t, 2 * t + 1):
                    o = (n % 2) * 864
                    psos = []
                    for cc in range(2):
                        psos.append(avp.tile([128, 288], F32, tag="av",
                                             name=f"pso{b}_{n}_{cc}"))
                    for k in range(3):
                        for cc in range(2):
                            np_ = 97 if k == 0 else 96
                            nc.tensor.matmul(
                                psos[cc][:],
                                _apvp(v, 0, np_, (3 * n + k) * 256 + cc * 128,
                                      [[1, 128]]),
                                _apvp(rhs, 0, np_, o + k * 288, [[1, 288]]),
                                start=(k == 0), stop=(k == 2),
                            )
                    for cc in range(2):
                        t8 = st[b][f"t8_{cc}_{n // 8}"]
                        nc.scalar.copy(
                            t8[:, (n % 8) * 288:(n % 8) * 288 + 288],
                            psos[cc][:])

            def stage_tile_fini(b, t):
                # add staged attention of tile t onto xs (2 bands = 576 cols)
                xs = st[b]["xs"]
                grp = (2 * t) // 8
                c0 = (2 * t) % 8 * 288
                for cc in range(2):
                    t8 = st[b][f"t8_{cc}_{grp}"]
                    nc.vector.tensor_tensor(
                        xs[cc][:, 576 * t:576 * t + 576],
                        xs[cc][:, 576 * t:576 * t + 576],
                        t8[:, c0:c0 + 576],
                        ADD,
                    )

            def stage_store(b, half):
                h0 = half * 4608
                for cc in range(2):
                    nc.sync.dma_start(
                        out_e[b, cc, :, h0:h0 + 4608],
                        st[b]["xs"][cc][:, h0:h0 + 4608],
                    )

            def alloc_t8(b):
                for grp in range(4):
                    for cc in range(2):
                        st[b][f"t8_{cc}_{grp}"] = wp.tile(
                            [128, 2304], BF16, tag="t8", bufs=2,
                            name=f"t8_{b}_{cc}_{grp}")

            # ================= emission schedule =================
            stage_load(0)
            for g in range(96):
                stage_proj_group(0, g)
            stage_ones(0)
            stage_transp(0)

            stage_load(1)
            # proj(1) on PE while vector/gpsimd do scores(0)
            for g in range(16):
                stage_proj_group(1, g)
            stage_scores_dir(0, "v", 0)
            for g in range(16, 32):
                stage_proj_group(1, g)
            stage_scores_dir(0, "v", 1)
            for g in range(32, 48):
                stage_proj_group(1, g)
            stage_avt(0)            # PE transposes mid proj(1)
            stage_scores_dir(0, "h", 0)
            for g in range(48, 64):
                stage_proj_group(1, g)
            stage_scores_dir(0, "h", 1)
            for g in range(64, 96):
                stage_proj_group(1, g)
            stage_btt(0)
            stage_ones(1)
            stage_transp(1)

            alloc_t8(0)
            # AV(0) with rhs built one tile ahead; scores(1) interleaved
            rhs_q = [stage_rhs(0, 0), stage_rhs(0, 1)]
            for t in range(16):
                rhs = rhs_q.pop(0)
                if t < 14:
                    rhs_q.append(stage_rhs(0, t + 2))
                if t == 0:
                    stage_scores_dir(1, "v", 0)
                if t == 2:
                    stage_scores_dir(1, "v", 1)
                stage_av_tile(0, t, rhs)
                if t > 0:
                    stage_tile_fini(0, t - 1)
                if t == 15:
                    stage_tile_fini(0, 15)
                    stage_store(0, 0)
                    stage_store(0, 1)
                if t == 6:
                    stage_avt(1)
                    stage_scores_dir(1, "h", 0)
                if t == 9:
                    stage_scores_dir(1, "h", 1)
                if t == 12:
                    stage_btt(1)

            alloc_t8(1)
            rhs_q = [stage_rhs(1, 0), stage_rhs(1, 1)]
            for t in range(16):
                rhs = rhs_q.pop(0)
                if t < 14:
                    rhs_q.append(stage_rhs(1, t + 2))
                stage_av_tile(1, t, rhs)
                if t > 0:
                    stage_tile_fini(1, t - 1)
                if t == 15:
                    stage_tile_fini(1, 15)
                    stage_store(1, 0)
                    stage_store(1, 1)
    nc.compile()
    return nc


def _host_prep(x, Wq, bq, Wk, bk, Wv, bv, gamma):
    x = np.ascontiguousarray(x, np.float32)
    g = float(np.asarray(gamma).reshape(-1)[0])
    sig_w = (bq @ Wk).astype(np.float32)
    z1 = np.zeros((1, 256), np.float32)
    wall = np.concatenate([Wq, z1, Wk, sig_w[None], Wv * g], 0)  # [322,256]
    wallT = np.stack([np.ascontiguousarray(wall[:, :128].T),
                      np.ascontiguousarray(wall[:, 128:].T)])
    ipat2 = np.tile(np.eye(96), (1, 18)).astype(ml_dtypes.bfloat16)
    pstr = np.kron(np.eye(32), np.ones((3, 3))).astype(ml_dtypes.bfloat16)
    idtb = np.eye(96).astype(ml_dtypes.bfloat16)
    bvrow = np.tile(bv.astype(np.float32) * g, 96)[None, :].astype(
        ml_dtypes.float8_e4m3)
    xr = x.reshape(B, 2, 128, S)
    in_maps = []
    for i in range(NCORE):
        in_maps.append({
            "xa": np.ascontiguousarray(
                xr[i * BPC:(i + 1) * BPC]).astype(ml_dtypes.bfloat16),
            "wall": wallT.astype(ml_dtypes.bfloat16),
            "ipat2": ipat2, "pstr": pstr, "idtb": idtb, "bvrow": bvrow,
        })
    return in_maps


_CACHE = {}


def kernel(x, Wq, bq, Wk, bk, Wv, bv, gamma, _trace=False):
    x = np.asarray(x, np.float32)
    in_maps = _host_prep(x, np.asarray(Wq, np.float32),
                         np.asarray(bq, np.float32),
                         np.asarray(Wk, np.float32),
                         np.asarray(bk, np.float32),
                         np.asarray(Wv, np.float32),
                         np.asarray(bv, np.float32),
                         np.asarray(gamma, np.float32))
    if "nc" not in _CACHE:
        _CACHE["nc"] = build_graph()
    nc = _CACHE["nc"]
    res = run_bass_kernel_spmd(nc, in_maps, list(range(NCORE)), trace=_trace)
    kernel.last_result = res
    out = np.empty((B, C, H, W), np.float32)
    for i in range(NCORE):
        o = np.asarray(res.results[i]["out"], np.float32)
        for b in range(BPC):
            out[i * BPC + b] = o[b].reshape(C, H, W)
    return out


if __name__ == "__main__":
    rng = np.random.default_rng(0)
    xs = {k: rng.standard_normal(s).astype(np.float32) * (0.05 if k != "x" else 1.0)
          for k, s in [("x", (16, 256, 96, 96)), ("Wq", (32, 256)), ("bq", (32,)),
                       ("Wk", (32, 256)), ("bk", (32,)), ("Wv", (256, 256)),
                       ("bv", (256,)), ("gamma", (1,))]}
    y = kernel(**xs)
    print("ran", y.shape)


# revision 6
# speedup vs baseline: 1.0747x; 1.0032x over previous
import sys

for p in ("/opt/trn_rl_repo",):
    if p not in sys.path:
        sys.path.insert(0, p)

import numpy as np
import ml_dtypes

from concourse import bass, mybir, bacc, tile
from concourse.ap import AP
from concourse.bass_utils import run_bass_kernel_spmd


def _install_ntff_hook():
    try:
        from antenv import axon_hooks  # noqa: F401
        return
    except ImportError:
        pass
    import types
    try:
        import antenv
    except ImportError:
        return
    mod = types.ModuleType("antenv.axon_hooks")
    _h = {"hook": None}
    mod.set_axon_ntff_profile_hook = lambda h: _h.__setitem__("hook", h)
    mod.get_axon_ntff_profile_hook = lambda: _h["hook"]
    sys.modules["antenv.axon_hooks"] = mod
    antenv.axon_hooks = mod
    try:
        from trn_agent_boot.trn_boot import _ntff_profile_via_ctypes
        h = _ntff_profile_via_ctypes("/opt/axon/libaxon_pjrt.so")
        if h is not None:
            mod.set_axon_ntff_profile_hook(h)
    except Exception:
        pass


_install_ntff_hook()


def _enable_ldw_opt():
    import concourse.bass_utils as _bu
    if getattr(_bu, "_ldw_patched", False):
        return
    _orig = _bu.run_command

    def _patched(argv, **kw):
        try:
            argv = ["--enable-ldw-opt=true" if c == "--enable-ldw-opt=false" else c
                    for c in argv]
        except TypeError:
            pass
        return _orig(argv, **kw)

    _bu.run_command = _patched
    _bu._ldw_patched = True


# _enable_ldw_opt() breaks walrus codegen (InstLdweights incompatible)

F32 = mybir.dt.float32
BF16 = mybir.dt.bfloat16
F8 = mybir.dt.float8e4
MUL = mybir.AluOpType.mult
ADD = mybir.AluOpType.add
AXX = mybir.AxisListType.X
EXP = mybir.ActivationFunctionType.Exp

B, C, H, W = 16, 256, 96, 96
S = H * W
NB = 32
NCORE = 8
BPC = B // NCORE
QKW = 66            # ch0..31 q, 32 ones, 33..64 k, 65 sigma (no pad)
PW = QKW + 256
NCHT = 66           # transposed channels 0..65

V_DT = F8           # value matrix storage dtype
RHS_DT = F8         # rhs (attention pattern) dtype — must match v dtype for PE


def _apv(t, off, dims):
    b = t[:] if not isinstance(t, AP) else t
    part = list(b.ap[0])
    return AP(b.tensor, b.offset + off, [part] + [list(d) for d in dims])


def _apvp(t, p0, np_, off, dims):
    """View with custom partition range [p0, p0+np_) and custom free dims."""
    b = t[:] if not isinstance(t, AP) else t
    part = [b.ap[0][0], np_]
    return AP(b.tensor, b.offset + p0 * b.ap[0][0] + off,
              [part] + [list(d) for d in dims])


def build_graph():
    nc = bacc.Bacc(None, target_bir_lowering=False)

    xa_e = nc.declare_dram_parameter("xa", [BPC, 2, 128, S], BF16, isOutput=False)
    wall_e = nc.declare_dram_parameter("wall", [2, 128, PW], BF16, isOutput=False)
    ipat2_e = nc.declare_dram_parameter("ipat2", [96, 1728], BF16, isOutput=False)
    pstr_e = nc.declare_dram_parameter("pstr", [96, 96], BF16, isOutput=False)
    idtb_e = nc.declare_dram_parameter("idtb", [96, 96], BF16, isOutput=False)
    bvrow_e = nc.declare_dram_parameter("bvrow", [1, 96 * 256], V_DT, isOutput=False)
    out_e = nc.declare_dram_parameter("out", [BPC, 2, 128, S], BF16, isOutput=True)

    with tile.TileContext(nc) as tc, nc.allow_low_precision(reason="bf16 score accum ok for 2e-2 tol"):
        with (
            tc.tile_pool(name="const", bufs=1) as cp,
            tc.tile_pool(name="main", bufs=1) as mp,
            tc.tile_pool(name="work", bufs=2) as wp,
            tc.tile_pool(name="pj", bufs=5, space="PSUM") as pj,
            tc.tile_pool(name="avp", bufs=3, space="PSUM") as avp,
        ):
            wall_sb = []
            for cc in range(2):
                t = cp.tile([128, PW], BF16, tag=f"wall{cc}")
                nc.sync.dma_start(t[:], wall_e[cc])
                wall_sb.append(t)
            ipat2_sb = cp.tile([96, 1728], BF16, tag="ipat2")
            nc.sync.dma_start(ipat2_sb[:], ipat2_e[:])
            pstr_sb = cp.tile([96, 96], BF16, tag="pstr")
            nc.sync.dma_start(pstr_sb[:], pstr_e[:])
            idtb_sb = cp.tile([96, 96], BF16, tag="idtb")
            nc.sync.dma_start(idtb_sb[:], idtb_e[:])

            st = {}

            def stage_load(b):
                xs = []
                for cc in range(2):
                    t = mp.tile([128, S], BF16, tag=f"xa{cc}", bufs=2,
                                name=f"xa{cc}_{b}")
                    xs.append(t)
                # interleave cc chunks so early proj groups unblock fast
                for q in range(4):
                    for cc in range(2):
                        nc.sync.dma_start(xs[cc][:, q * 2304:(q + 1) * 2304],
                                          xa_e[b, cc, :, q * 2304:(q + 1) * 2304])
                v = mp.tile([97, 96 * 256], V_DT, tag="v", bufs=2, name=f"v{b}")
                nc.sync.dma_start(v[96:97, :], bvrow_e[:])
                qk = mp.tile([96, 96 * QKW], BF16, tag="qk", bufs=2, name=f"qk{b}")
                qkt = mp.tile([96, 96 * NCHT], BF16, tag="qkt", bufs=2,
                              name=f"qkt{b}")
                st[b] = {"xs": xs, "v": v, "qk": qk, "qkt": qkt}

            def stage_proj_group(b, g):
                # 1 line per group, g in 0..95
                xs, v, qk = st[b]["xs"], st[b]["v"], st[b]["qk"]
                ps = pj.tile([96, 512], F32, tag="ps", name=f"ps{b}_{g}")
                h = g
                for cc in range(2):
                    nc.tensor.matmul(
                        _apv(ps, 0, [[1, PW]]),
                        xs[cc][:, h * 96:(h + 1) * 96],
                        wall_sb[cc][:],
                        start=(cc == 0),
                        stop=(cc == 1),
                    )
                qk_dst = _apv(qk[:], g * QKW, [[1, QKW]])
                qk_src = _apv(ps, 0, [[1, QKW]])
                v_dst = _apvp(v, 0, 96, g * 256, [[1, 256]])
                v_src = _apv(ps, QKW, [[1, 256]])
                if g % 2 == 0:
                    nc.vector.tensor_copy(v_dst, v_src)
                    nc.scalar.copy(qk_dst, qk_src)
                else:
                    nc.scalar.copy(v_dst, v_src)
                    nc.vector.tensor_copy(qk_dst, qk_src)

            def stage_ones(b):
                # q channel 32 := 1.0 (pairs with sigma channel of k)
                nc.gpsimd.memset(_apv(st[b]["qk"][:], 32, [[QKW, 96]]), 1.0)

            def stage_transp(b):
                qk, qkt = st[b]["qk"], st[b]["qkt"]
                for grp in range(7):
                    ptq = pj.tile([96, 1024], BF16, tag="ps", name=f"ptq{b}_{grp}")
                    for i in range(min(10, NCHT - grp * 10)):
                        ch = grp * 10 + i
                        nc.tensor.transpose(
                            _apv(ptq, i * 96, [[1, 96]]),
                            _apv(qk[:], ch, [[QKW, 96]]),
                            idtb_sb[:],
                        )
                    nch = min(10, NCHT - grp * 10)
                    dst = _apv(qkt[:], grp * 10, [[NCHT, 96], [1, nch]])
                    srcv = _apv(ptq, 0, [[1, 96], [96, nch]])
                    if grp % 2 == 0:
                        nc.vector.tensor_copy(dst, srcv)
                    else:
                        nc.scalar.copy(dst, srcv)

            def stage_scores_dir(b, d, half):
                # d: 'v' or 'h'; half in (0,1): bands 16*half..16*half+15
                # layout te/A: [p, (3k+j)*32 + band]
                src = st[b]["qkt"] if d == "v" else st[b]["qk"]
                cw = NCHT if d == "v" else QKW
                if half == 0:
                    te = mp.tile([96, 288], BF16, tag=f"te{d}", bufs=1,
                                 name=f"te{d}{b}")
                    A = mp.tile([96, 288], BF16, tag=f"A{d}",
                                bufs=(1 if d == "v" else 2), name=f"A{d}{b}")
                    s3 = mp.tile([96, 96], BF16, tag=f"s3{d}", bufs=1,
                                 name=f"s3{d}{b}")
                    r3 = mp.tile([96, 96], BF16, tag=f"r3{d}", bufs=1,
                                 name=f"r3{d}{b}")
                    bs = mp.tile([96, 96], BF16, tag=f"bs{d}", bufs=1,
                                 name=f"bs{d}{b}")
                    st[b]["te" + d], st[b]["A" + d] = te, A
                    st[b]["s3" + d], st[b]["r3" + d] = s3, r3
                    st[b]["bs" + d] = bs
                te, A = st[b]["te" + d], st[b]["A" + d]
                s3, r3 = st[b]["s3" + d], st[b]["r3" + d]
                bs = st[b]["bs" + d]
                h0 = 16 * half
                for kj in range(9):
                    k, j = divmod(kj, 3)
                    prod = wp.tile([96, 512], BF16, tag="prod", bufs=4)
                    if b == 0:
                        peng = nc.gpsimd
                    else:
                        peng = nc.vector if kj % 3 != 2 else nc.gpsimd
                    peng.tensor_tensor(
                        prod[:, 0:512],
                        _apv(src[:], k * cw + h0 * 3 * cw, [[3 * cw, 16], [1, 32]]),
                        _apv(src[:], j * cw + 33 + h0 * 3 * cw,
                             [[3 * cw, 16], [1, 32]]),
                        MUL,
                    )
                    nc.vector.tensor_reduce(
                        te[:, kj * 32 + h0:kj * 32 + h0 + 16],
                        _apv(prod[:], 0, [[32, 16], [1, 32]]),
                        AXX, ADD,
                    )
                # add sigma (j-pixel bias channel 65) before exp
                nc.vector.tensor_tensor(
                    _apv(te[:], h0, [[96, 3], [32, 3], [1, 16]]),
                    _apv(te[:], h0, [[96, 3], [32, 3], [1, 16]]),
                    _apv(src[:], 65 + h0 * 3 * cw, [[0, 3], [cw, 3], [3 * cw, 16]]),
                    ADD,
                )
                nc.scalar.activation(
                    _apv(te[:], h0, [[32, 9], [1, 16]]),
                    _apv(te[:], h0, [[32, 9], [1, 16]]), EXP)
                nc.vector.tensor_reduce(
                    _apv(s3[:], h0, [[32, 3], [1, 16]]),
                    _apv(te[:], h0, [[96, 3], [1, 16], [32, 3]]),
                    AXX, ADD,
                )
                nc.vector.reciprocal(
                    _apv(r3[:], h0, [[32, 3], [1, 16]]),
                    _apv(s3[:], h0, [[32, 3], [1, 16]]))
                nc.vector.tensor_tensor(
                    _apv(A[:], h0, [[96, 3], [32, 3], [1, 16]]),
                    _apv(te[:], h0, [[96, 3], [32, 3], [1, 16]]),
                    _apv(r3[:], h0, [[32, 3], [0, 3], [1, 16]]),
                    MUL,
                )
                # bias partial sums: sum_k A[k,j] -> bs[p, 3n+j]
                nc.vector.tensor_reduce(
                    _apv(bs[:], 3 * h0, [[3, 16], [1, 3]]),
                    _apv(A[:], h0, [[1, 16], [32, 3], [96, 3]]),
                    AXX, ADD,
                )

            def stage_avt(b):
                # av_p[h, 96j + (3m+k)] = A_v[h, (3k+j)*32+m]
                A = st[b]["Av"]
                av_p = mp.tile([96, 288], BF16, tag="avp2", bufs=1,
                               name=f"avp2{b}")
                nc.vector.tensor_copy(
                    _apv(av_p[:], 0, [[96, 3], [3, 32], [1, 3]]),
                    _apv(A[:], 0, [[32, 3], [1, 32], [96, 3]]),
                )
                avtJ = mp.tile([96, 288], BF16, tag="avtJ", bufs=2,
                               name=f"avtJ{b}")
                for j in range(3):
                    pt = avp.tile([128, 288], BF16, tag="av", name=f"avt{b}_{j}")
                    nc.tensor.transpose(
                        _apvp(pt, 0, 96, 0, [[1, 96]]),
                        av_p[:, j * 96:(j + 1) * 96],
                        idtb_sb[:],
                    )
                    nc.vector.tensor_copy(
                        _apv(avtJ[:], j, [[3, 96]]),
                        _apvp(pt, 0, 96, 0, [[1, 96]]),
                    )
                st[b]["avtJ"] = avtJ

            def stage_btt(b):
                # btT[h, w'] = bsh[w', h]^T + bsv[h, w']
                pt = avp.tile([128, 288], BF16, tag="av", name=f"btt{b}")
                nc.tensor.transpose(
                    _apvp(pt, 0, 96, 0, [[1, 96]]),
                    st[b]["bsh"][:],
                    idtb_sb[:],
                )
                btT = mp.tile([96, 96], RHS_DT, tag="btT", bufs=2, name=f"btT{b}")
                nc.vector.tensor_tensor(
                    btT[:],
                    _apvp(pt, 0, 96, 0, [[1, 96]]),
                    st[b]["bsv"][:],
                    ADD,
                )
                st[b]["btT"] = btT

            def stage_rhs(b, t):
                # rhs tile for bands 4t..4t+3:
                # rhs[w, n*864 + (3k+j)*96 + w'] , row 96 = bias coefs (k=0 slice)
                Ah, avtJ, btT = st[b]["Ah"], st[b]["avtJ"], st[b]["btT"]
                rhs = wp.tile([97, 1728], RHS_DT, tag="rhs", bufs=3, name=f"rhs{b}_{t}")
                eeng = nc.vector if t % 2 == 0 else nc.gpsimd
                eeng.tensor_tensor(
                    _apvp(rhs, 0, 96, 0, [[864, 2], [96, 9], [1, 96]]),
                    _apv(ipat2_sb[:], 0, [[864, 2], [96, 9], [1, 96]]),
                    _apv(Ah[:], 2 * t, [[1, 2], [32, 9], [0, 96]]),
                    MUL,
                )
                for n in range(2):
                    tmp = wp.tile([96, 288], BF16, tag="vtmp", bufs=4)
                    teng = nc.vector if t % 2 == 0 else nc.gpsimd
                    teng.tensor_tensor(
                        _apv(tmp[:], 0, [[96, 3], [3, 32], [1, 3]]),
                        _apv(pstr_sb[:], 0, [[0, 3], [3, 32], [1, 3]]),
                        _apv(avtJ[:], (6 * t + 3 * n) * 3,
                             [[3, 3], [0, 32], [1, 3]]),
                        MUL,
                    )
                    teng.tensor_tensor(
                        _apvp(rhs, 0, 96, n * 864, [[384, 3], [1, 96]]),
                        _apvp(rhs, 0, 96, n * 864, [[384, 3], [1, 96]]),
                        _apv(tmp[:], 0, [[96, 3], [1, 96]]),
                        ADD,
                    )
                # bias row: btT rows -> row 96, k=0 slice per band
                for n in range(2):
                    nc.sync.dma_start(
                        _apvp(rhs, 96, 1, n * 864, [[96, 3], [1, 96]]),
                        btT[6 * t + 3 * n:6 * t + 3 * n + 3, :],
                    )
                return rhs

            def stage_av_tile(b, t, rhs):
                v = st[b]["v"]
                for n in (2 * t, 2 * t + 1):
                    o = (n % 2) * 864
                    psos = []
                    for cc in range(2):
                        psos.append(avp.tile([128, 288], F32, tag="av",
                                             name=f"pso{b}_{n}_{cc}"))
                    for k in range(3):
                        for cc in range(2):
                            np_ = 97 if k == 0 else 96
                            nc.tensor.matmul(
                                psos[cc][:],
                                _apvp(v, 0, np_, (3 * n + k) * 256 + cc * 128,
                                      [[1, 128]]),
                                _apvp(rhs, 0, np_, o + k * 288, [[1, 288]]),
                                start=(k == 0), stop=(k == 2),
                            )
                    for cc in range(2):
                        t8 = st[b][f"t8_{cc}_{n // 8}"]
                        nc.scalar.copy(
                            t8[:, (n % 8) * 288:(n % 8) * 288 + 288],
                            psos[cc][:])

            def stage_tile_fini(b, t):
                # add staged attention of tile t onto xs (2 bands = 576 cols)
                xs = st[b]["xs"]
                grp = (2 * t) // 8
                c0 = (2 * t) % 8 * 288
                for cc in range(2):
                    t8 = st[b][f"t8_{cc}_{grp}"]
                    nc.vector.tensor_tensor(
                        xs[cc][:, 576 * t:576 * t + 576],
                        xs[cc][:, 576 * t:576 * t + 576],
                        t8[:, c0:c0 + 576],
                        ADD,
                    )

            def stage_store(b, half):
                h0 = half * 4608
                for cc in range(2):
                    nc.sync.dma_start(
                        out_e[b, cc, :, h0:h0 + 4608],
                        st[b]["xs"][cc][:, h0:h0 + 4608],
                    )

            def alloc_t8(b):
                for grp in range(4):
                    for cc in range(2):
                        st[b][f"t8_{cc}_{grp}"] = wp.tile(
                            [128, 2304], BF16, tag="t8", bufs=2,
                            name=f"t8_{b}_{cc}_{grp}")

            # ================= emission schedule =================
            stage_load(0)
            for g in range(96):
                stage_proj_group(0, g)
            stage_ones(0)
            stage_transp(0)

            stage_load(1)
            # proj(1) on PE while vector/gpsimd do scores(0)
            for g in range(16):
                stage_proj_group(1, g)
            stage_scores_dir(0, "v", 0)
            for g in range(16, 32):
                stage_proj_group(1, g)
            stage_scores_dir(0, "v", 1)
            for g in range(32, 48):
                stage_proj_group(1, g)
            stage_avt(0)            # PE transposes mid proj(1)
            stage_scores_dir(0, "h", 0)
            for g in range(48, 64):
                stage_proj_group(1, g)
            stage_scores_dir(0, "h", 1)
            for g in range(64, 96):
                stage_proj_group(1, g)
            stage_btt(0)
            stage_ones(1)
            stage_transp(1)

            alloc_t8(0)
            # AV(0) with rhs built one tile ahead; scores(1) interleaved
            rhs_q = [stage_rhs(0, 0), stage_rhs(0, 1)]
            for t in range(16):
                rhs = rhs_q.pop(0)
                if t < 14:
                    rhs_q.append(stage_rhs(0, t + 2))
                if t == 0:
                    stage_scores_dir(1, "v", 0)
                if t == 2:
                    stage_scores_dir(1, "v", 1)
                stage_av_tile(0, t, rhs)
                if t > 0:
                    stage_tile_fini(0, t - 1)
                if t == 15:
                    stage_tile_fini(0, 15)
                    stage_store(0, 0)
                    stage_store(0, 1)
                if t == 6:
                    stage_avt(1)
                    stage_scores_dir(1, "h", 0)
                if t == 9:
                    stage_scores_dir(1, "h", 1)
                if t == 12:
                    stage_btt(1)

            alloc_t8(1)
            rhs_q = [stage_rhs(1, 0), stage_rhs(1, 1)]
            for t in range(16):
                rhs = rhs_q.pop(0)
                if t < 14:
                    rhs_q.append(stage_rhs(1, t + 2))
                stage_av_tile(1, t, rhs)
                if t > 0:
                    stage_tile_fini(1, t - 1)
                if t == 15:
                    stage_tile_fini(1, 15)
                    stage_store(1, 0)
                    stage_store(1, 1)
    nc.compile()
    return nc


def _host_prep(x, Wq, bq, Wk, bk, Wv, bv, gamma):
    x = np.ascontiguousarray(x, np.float32)
    g = float(np.asarray(gamma).reshape(-1)[0])
    sig_w = (bq @ Wk).astype(np.float32)
    z1 = np.zeros((1, 256), np.float32)
    wall = np.concatenate([Wq, z1, Wk, sig_w[None], Wv * g], 0)  # [322,256]
    wallT = np.stack([np.ascontiguousarray(wall[:, :128].T),
                      np.ascontiguousarray(wall[:, 128:].T)])
    ipat2 = np.tile(np.eye(96), (1, 18)).astype(ml_dtypes.bfloat16)
    pstr = np.kron(np.eye(32), np.ones((3, 3))).astype(ml_dtypes.bfloat16)
    idtb = np.eye(96).astype(ml_dtypes.bfloat16)
    bvrow = np.tile(bv.astype(np.float32) * g, 96)[None, :].astype(
        ml_dtypes.float8_e4m3)
    xr = x.reshape(B, 2, 128, S)
    in_maps = []
    for i in range(NCORE):
        in_maps.append({
            "xa": np.ascontiguousarray(
                xr[i * BPC:(i + 1) * BPC]).astype(ml_dtypes.bfloat16),
            "wall": wallT.astype(ml_dtypes.bfloat16),
            "ipat2": ipat2, "pstr": pstr, "idtb": idtb, "bvrow": bvrow,
        })
    return in_maps


_CACHE = {}


def kernel(x, Wq, bq, Wk, bk, Wv, bv, gamma, _trace=False):
    x = np.asarray(x, np.float32)
    in_maps = _host_prep(x, np.asarray(Wq, np.float32),
                         np.asarray(bq, np.float32),
                         np.asarray(Wk, np.float32),
                         np.asarray(bk, np.float32),
                         np.asarray(Wv, np.float32),
                         np.asarray(bv, np.float32),
                         np.asarray(gamma, np.float32))
    if "nc" not in _CACHE:
        _CACHE["nc"] = build_graph()
    nc = _CACHE["nc"]
    res = run_bass_kernel_spmd(nc, in_maps, list(range(NCORE)), trace=_trace)
    kernel.last_result = res
    out = np.empty((B, C, H, W), np.float32)
    for i in range(NCORE):
        o = np.asarray(res.results[i]["out"], np.float32)
        for b in range(BPC):
            out[i * BPC + b] = o[b].reshape(C, H, W)
    return out


if __name__ == "__main__":
    rng = np.random.default_rng(0)
    xs = {k: rng.standard_normal(s).astype(np.float32) * (0.05 if k != "x" else 1.0)
          for k, s in [("x", (16, 256, 96, 96)), ("Wq", (32, 256)), ("bq", (32,)),
                       ("Wk", (32, 256)), ("bk", (32,)), ("Wv", (256, 256)),
                       ("bv", (256,)), ("gamma", (1,))]}
    y = kernel(**xs)
    print("ran", y.shape)
